# revision 2
# baseline (speedup 1.0000x reference)
import sys

import numpy as np

for p in ("/opt/trn_rl_repo",):
    if p not in sys.path:
        sys.path.insert(0, p)

import concourse.bass as bass  # noqa: E402
import concourse.tile as tile  # noqa: E402
from concourse import bacc, mybir  # noqa: E402
from concourse.bass_utils import run_bass_kernel_spmd  # noqa: E402

B, N, D = 128, 512, 512
NCORES = 8
BPC = B // NCORES  # 16 batch items per core
F32 = mybir.dt.float32


def _hadamard(n: int) -> np.ndarray:
    H = np.array([[1.0]], dtype=np.float32)
    base = np.array([[1.0, 1.0], [1.0, -1.0]], dtype=np.float32)
    while H.shape[0] < n:
        H = np.kron(H, base)
    return H


def _build():
    nc = bacc.Bacc("TRN2", target_bir_lowering=False, debug=False)
    x_d = nc.dram_tensor("x", [BPC, N, D], F32, kind="ExternalInput").ap()
    h_d = nc.dram_tensor("h", [128, 4 * N], F32, kind="ExternalInput").ap()
    hs_d = nc.dram_tensor("hs", [128, 4 * N], F32, kind="ExternalInput").ap()
    y_d = nc.dram_tensor("y", [BPC, N, D], F32, kind="ExternalOutput").ap()

    with tile.TileContext(nc) as tc:
        with (
            tc.tile_pool(name="const", bufs=1) as const_pool,
            tc.tile_pool(name="xp", bufs=3) as x_pool,
            tc.tile_pool(name="tp", bufs=2) as t_pool,
            tc.tile_pool(name="op", bufs=8) as o_pool,
            tc.tile_pool(name="ps", bufs=8, space="PSUM") as psum_pool,
        ):
            # H laid out as [128, c*512+n] with row m = c*128+p
            h_sb = const_pool.tile([128, 4 * N], F32)
            nc.sync.dma_start(h_sb[:], h_d[:])
            hs_sb = const_pool.tile([128, 4 * N], F32, tag="hs")
            nc.sync.dma_start(hs_sb[:], hs_d[:])

            for b in range(BPC):
                xt = x_pool.tile([128, 4 * D], F32)
                for c in range(4):
                    nc.sync.dma_start(
                        xt[:, c * D : (c + 1) * D],
                        x_d[b, c * 128 : (c + 1) * 128, :],
                    )
                # t_T[d, n] = sum_m x[m, d] * H[m, n]  (H symmetric)
                tt = t_pool.tile([128, 4 * N], F32)
                for dt_ in range(4):
                    ps = psum_pool.tile([128, N], F32)
                    for kc in range(4):
                        nc.tensor.matmul(
                            ps[:],
                            xt[:, kc * D + dt_ * 128 : kc * D + dt_ * 128 + 128],
                            h_sb[:, kc * N : (kc + 1) * N],
                            start=(kc == 0),
                            stop=(kc == 3),
                        )
                    nc.any.tensor_copy(tt[:, dt_ * N : (dt_ + 1) * N], ps[:])
                # y[n, e] = sum_d t_T[d, n] * (H/512)[d, e]
                for nt in range(4):
                    ps = psum_pool.tile([128, D], F32)
                    for dc in range(4):
                        nc.tensor.matmul(
                            ps[:],
                            tt[:, dc * N + nt * 128 : dc * N + nt * 128 + 128],
                            hs_sb[:, dc * D : (dc + 1) * D],
                            start=(dc == 0),
                            stop=(dc == 3),
                        )
                    ot = o_pool.tile([128, D], F32)
                    nc.any.tensor_copy(ot[:], ps[:])
                    nc.sync.dma_start(y_d[b, nt * 128 : (nt + 1) * 128, :], ot[:])

    nc.compile()
    return nc


_NC = None


def _in_maps(x: np.ndarray) -> list[dict]:
    H = _hadamard(N)
    # layout [128, c*512+n] with row m = c*128+p
    h_l = np.ascontiguousarray(
        H.reshape(4, 128, N).transpose(1, 0, 2).reshape(128, 4 * N)
    )
    hs_l = np.ascontiguousarray(h_l / np.float32(512.0))
    return [
        {"x": x[i * BPC : (i + 1) * BPC], "h": h_l, "hs": hs_l}
        for i in range(NCORES)
    ]


def _gather(results: list[dict]) -> np.ndarray:
    return np.concatenate([r["y"] for r in results], axis=0).astype(np.float32)


def kernel(x: np.ndarray) -> np.ndarray:
    global _NC
    if _NC is None:
        _NC = _build()
    x = np.ascontiguousarray(np.asarray(x), dtype=np.float32)
    res = run_bass_kernel_spmd(_NC, _in_maps(x), list(range(NCORES))).results
    return _gather(res)



# revision 8
# speedup vs baseline: 2.8345x; 2.8345x over previous
import sys

import numpy as np

for p in ("/opt/trn_rl_repo",):
    if p not in sys.path:
        sys.path.insert(0, p)

import concourse.bass as bass  # noqa: E402
import concourse.tile as tile  # noqa: E402
from concourse import bacc, mybir  # noqa: E402
from concourse.bass_utils import run_bass_kernel_spmd  # noqa: E402

B, N, D = 128, 512, 512
NCORES = 8
BPC = B // NCORES  # 16 batch items per core
F32 = mybir.dt.float32
BF16 = mybir.dt.bfloat16


def _hadamard(n: int) -> np.ndarray:
    H = np.array([[1.0]], dtype=np.float32)
    base = np.array([[1.0, 1.0], [1.0, -1.0]], dtype=np.float32)
    while H.shape[0] < n:
        H = np.kron(H, base)
    return H


def _build():
    nc = bacc.Bacc("TRN2", target_bir_lowering=False, debug=False)
    x_d = nc.dram_tensor("x", [BPC, N, D], F32, kind="ExternalInput").ap()
    h_d = nc.dram_tensor("h", [128, 4 * N], BF16, kind="ExternalInput").ap()
    hs_d = nc.dram_tensor("hs", [128, 4 * N], BF16, kind="ExternalInput").ap()
    y_d = nc.dram_tensor("y", [BPC, N, D], F32, kind="ExternalOutput").ap()

    with tile.TileContext(nc) as tc:
        with (
            tc.tile_pool(name="const", bufs=1) as const_pool,
            tc.tile_pool(name="xf", bufs=2) as xf_pool,
            tc.tile_pool(name="xb", bufs=2) as xb_pool,
            tc.tile_pool(name="tp", bufs=2) as t_pool,
            tc.tile_pool(name="op", bufs=8) as o_pool,
            tc.tile_pool(name="ps", bufs=8, space="PSUM") as psum_pool,
        ):
            # H laid out as [128, c*512+n] with row m = c*128+p
            h_sb = const_pool.tile([128, 4 * N], BF16)
            nc.sync.dma_start(h_sb[:], h_d[:])
            hs_sb = const_pool.tile([128, 4 * N], BF16, tag="hs")
            nc.sync.dma_start(hs_sb[:], hs_d[:])

            for b in range(BPC):
                xf = xf_pool.tile([128, 4 * D], F32)
                for c in range(4):
                    nc.sync.dma_start(
                        xf[:, c * D : (c + 1) * D],
                        x_d[b, c * 128 : (c + 1) * 128, :],
                    )
                # cast to bf16 on gpsimd (SBUF->SBUF; gpsimd cannot touch PSUM)
                xb = xb_pool.tile([128, 4 * D], BF16)
                nc.gpsimd.tensor_copy(xb[:], xf[:])
                # t_T[d, n] = sum_m x[m, d] * H[m, n]  (H symmetric)
                tt = t_pool.tile([128, 4 * N], BF16)
                for dt_ in range(4):
                    ps = psum_pool.tile([128, N], F32)
                    for kc in range(4):
                        nc.tensor.matmul(
                            ps[:],
                            xb[:, kc * D + dt_ * 128 : kc * D + dt_ * 128 + 128],
                            h_sb[:, kc * N : (kc + 1) * N],
                            start=(kc == 0),
                            stop=(kc == 3),
                        )
                    nc.vector.tensor_copy(tt[:, dt_ * N : (dt_ + 1) * N], ps[:])
                # y[n, e] = sum_d t_T[d, n] * (H/512)[d, e]
                for nt in range(4):
                    ps = psum_pool.tile([128, D], F32)
                    for dc in range(4):
                        nc.tensor.matmul(
                            ps[:],
                            tt[:, dc * N + nt * 128 : dc * N + nt * 128 + 128],
                            hs_sb[:, dc * D : (dc + 1) * D],
                            start=(dc == 0),
                            stop=(dc == 3),
                        )
                    ot = o_pool.tile([128, D], F32)
                    nc.scalar.copy(ot[:], ps[:])
                    nc.sync.dma_start(y_d[b, nt * 128 : (nt + 1) * 128, :], ot[:])

    nc.compile()
    return nc


_NC = None


def _in_maps(x: np.ndarray) -> list[dict]:
    import ml_dtypes

    H = _hadamard(N)
    # layout [128, c*512+n] with row m = c*128+p
    h_l = np.ascontiguousarray(
        H.reshape(4, 128, N).transpose(1, 0, 2).reshape(128, 4 * N)
    ).astype(ml_dtypes.bfloat16)
    hs_l = np.ascontiguousarray(
        (H / np.float32(512.0))
        .reshape(4, 128, N)
        .transpose(1, 0, 2)
        .reshape(128, 4 * N)
    ).astype(ml_dtypes.bfloat16)
    return [
        {"x": x[i * BPC : (i + 1) * BPC], "h": h_l, "hs": hs_l}
        for i in range(NCORES)
    ]


def _gather(results: list[dict]) -> np.ndarray:
    return np.concatenate([r["y"] for r in results], axis=0).astype(np.float32)


def kernel(x: np.ndarray) -> np.ndarray:
    global _NC
    if _NC is None:
        _NC = _build()
    x = np.ascontiguousarray(np.asarray(x), dtype=np.float32)
    res = run_bass_kernel_spmd(_NC, _in_maps(x), list(range(NCORES))).results
    return _gather(res)


# revision 13
# speedup vs baseline: 4.9519x; 1.7470x over previous
import sys

import numpy as np

for p in ("/opt/trn_rl_repo",):
    if p not in sys.path:
        sys.path.insert(0, p)

import concourse.bass as bass  # noqa: E402
import concourse.tile as tile  # noqa: E402
from concourse import bacc, mybir  # noqa: E402
from concourse.bass_utils import run_bass_kernel_spmd  # noqa: E402

B, N, D = 128, 512, 512
NCORES = 8
BPC = B // NCORES  # 16 batch items per core
F32 = mybir.dt.float32
BF16 = mybir.dt.bfloat16


def _hadamard(n: int) -> np.ndarray:
    H = np.array([[1.0]], dtype=np.float32)
    base = np.array([[1.0, 1.0], [1.0, -1.0]], dtype=np.float32)
    while H.shape[0] < n:
        H = np.kron(H, base)
    return H


def _build():
    nc = bacc.Bacc("TRN2", target_bir_lowering=False, debug=False)
    x_d = nc.dram_tensor("x", [BPC, N, D], BF16, kind="ExternalInput").ap()
    h_d = nc.dram_tensor("h", [128, 4 * N], BF16, kind="ExternalInput").ap()
    hs_d = nc.dram_tensor("hs", [128, 4 * N], BF16, kind="ExternalInput").ap()
    y_d = nc.dram_tensor("y", [BPC, N, D], BF16, kind="ExternalOutput").ap()

    with tile.TileContext(nc) as tc:
        with (
            tc.tile_pool(name="const", bufs=1) as const_pool,
            tc.tile_pool(name="xb", bufs=2) as xb_pool,
            tc.tile_pool(name="tp", bufs=2) as t_pool,
            tc.tile_pool(name="op", bufs=8) as o_pool,
            tc.tile_pool(name="ps", bufs=8, space="PSUM") as psum_pool,
        ):
            # H laid out as [128, c*512+n] with row m = c*128+p
            h_sb = const_pool.tile([128, 4 * N], BF16)
            nc.sync.dma_start(h_sb[:], h_d[:])
            hs_sb = const_pool.tile([128, 4 * N], BF16, tag="hs")
            nc.sync.dma_start(hs_sb[:], hs_d[:])

            for b in range(BPC):
                xb = xb_pool.tile([128, 4 * D], BF16)
                for c in range(4):
                    nc.sync.dma_start(
                        xb[:, c * D : (c + 1) * D],
                        x_d[b, c * 128 : (c + 1) * 128, :],
                    )
                # t_T[d, n] = sum_m x[m, d] * H[m, n]  (H symmetric)
                tt = t_pool.tile([128, 4 * N], BF16)
                for dt_ in range(4):
                    ps = psum_pool.tile([128, N], F32)
                    for kc in range(4):
                        nc.tensor.matmul(
                            ps[:],
                            xb[:, kc * D + dt_ * 128 : kc * D + dt_ * 128 + 128],
                            h_sb[:, kc * N : (kc + 1) * N],
                            start=(kc == 0),
                            stop=(kc == 3),
                        )
                    nc.vector.tensor_copy(tt[:, dt_ * N : (dt_ + 1) * N], ps[:])
                # y[n, e] = sum_d t_T[d, n] * (H/512)[d, e]
                for nt in range(4):
                    ps = psum_pool.tile([128, D], F32)
                    for dc in range(4):
                        nc.tensor.matmul(
                            ps[:],
                            tt[:, dc * N + nt * 128 : dc * N + nt * 128 + 128],
                            hs_sb[:, dc * D : (dc + 1) * D],
                            start=(dc == 0),
                            stop=(dc == 3),
                        )
                    ot = o_pool.tile([128, D], BF16)
                    nc.scalar.copy(ot[:], ps[:])
                    nc.sync.dma_start(y_d[b, nt * 128 : (nt + 1) * 128, :], ot[:])

    nc.compile()
    return nc


_NC = None


def _in_maps(x: np.ndarray) -> list[dict]:
    import ml_dtypes

    H = _hadamard(N)
    # layout [128, c*512+n] with row m = c*128+p
    h_l = np.ascontiguousarray(
        H.reshape(4, 128, N).transpose(1, 0, 2).reshape(128, 4 * N)
    ).astype(ml_dtypes.bfloat16)
    hs_l = np.ascontiguousarray(
        (H / np.float32(512.0))
        .reshape(4, 128, N)
        .transpose(1, 0, 2)
        .reshape(128, 4 * N)
    ).astype(ml_dtypes.bfloat16)
    xb = x.astype(ml_dtypes.bfloat16)
    return [
        {"x": xb[i * BPC : (i + 1) * BPC], "h": h_l, "hs": hs_l}
        for i in range(NCORES)
    ]


def _gather(results: list[dict]) -> np.ndarray:
    return np.concatenate([r["y"] for r in results], axis=0).astype(np.float32)


def kernel(x: np.ndarray) -> np.ndarray:
    global _NC
    if _NC is None:
        _NC = _build()
    x = np.ascontiguousarray(np.asarray(x), dtype=np.float32)
    res = run_bass_kernel_spmd(_NC, _in_maps(x), list(range(NCORES))).results
    return _gather(res)


# revision 14
# speedup vs baseline: 5.3768x; 1.0858x over previous
"""Hadamard y = (1/512) H512 @ x @ H512 per batch item, 16 items/core.

Factorization: H512 = (H2 (x) I256) (I2 (x) H256)  [Sylvester, symmetric].
- column side: front butterfly over d-blocks (H2) + block-diag H256 matmul
- row side:    block-diag H256 matmul + back butterfly over row blocks (H2)
Tensor work/item: 32 MM of [128k,128m]x[128k,256n] (8192 rows) vs 16384
for full 512-matmuls. Butterflies ride DVE/Act. bf16 in/out (host casts).
"""
import sys

import numpy as np

for p in ("/opt/trn_rl_repo",):
    if p not in sys.path:
        sys.path.insert(0, p)

import concourse.bass as bass  # noqa: E402
import concourse.tile as tile  # noqa: E402
from concourse import bacc, mybir  # noqa: E402
from concourse.bass_utils import run_bass_kernel_spmd  # noqa: E402

B, N, D = 128, 512, 512
NCORES = 8
BPC = B // NCORES  # 16 batch items per core
F32 = mybir.dt.float32
BF16 = mybir.dt.bfloat16


def _hadamard(n: int) -> np.ndarray:
    H = np.array([[1.0]], dtype=np.float32)
    base = np.array([[1.0, 1.0], [1.0, -1.0]], dtype=np.float32)
    while H.shape[0] < n:
        H = np.kron(H, base)
    return H


def _build():
    nc = bacc.Bacc("TRN2", target_bir_lowering=False, debug=False)
    # x pre-relaid on host to the SBUF tile layout: x_t[b, p, c*512+d]
    # = x[b, c*128+p, d] -> one 4KB-per-partition DMA per item.
    x_d = nc.dram_tensor("x", [BPC, 128, 2048], BF16, kind="ExternalInput").ap()
    # H256 laid out [128, kq*256+nu] with row mu = kq*128+p
    h_d = nc.dram_tensor("h", [128, 512], BF16, kind="ExternalInput").ap()
    hs_d = nc.dram_tensor("hs", [128, 512], BF16, kind="ExternalInput").ap()
    # y left in tile layout: y_t[b, p, nb*1024+nq*512+e] = y[b, nb*256+nq*128+p, e]
    y_d = nc.dram_tensor("y", [BPC, 128, 2048], BF16, kind="ExternalOutput").ap()

    with tile.TileContext(nc) as tc:
        with (
            tc.tile_pool(name="const", bufs=1) as const_pool,
            tc.tile_pool(name="xb", bufs=3) as xb_pool,
            tc.tile_pool(name="wt", bufs=2) as wt_pool,
            tc.tile_pool(name="zb", bufs=2) as zb_pool,
            tc.tile_pool(name="yt", bufs=3) as yt_pool,
            tc.tile_pool(name="ps", bufs=1, space="PSUM") as psum_pool,
        ):
            h_sb = const_pool.tile([128, 512], BF16)
            nc.sync.dma_start(h_sb[:], h_d[:])
            hs_sb = const_pool.tile([128, 512], BF16, tag="hs")
            nc.sync.dma_start(hs_sb[:], hs_d[:])

            def front_half(b):
                """DMA in + front butterfly + MM1 + relayout copy -> zbig."""
                # xb[p, db*1024 + c*256 + du] = x[b, c*128+p, db*256+du];
                # row m=c*128+p (mb=c//2, mu-chunk kq=c%2); col d=(db, du).
                # db-halves contiguous so the butterfly uses flat APs (DVE
                # 16-bit fast mode needs contiguous patterns).
                xb = xb_pool.tile([128, 2048], BF16, name="xb")
                nc.sync.dma_start(xb[:], x_d[b])
                # A: front butterfly over column blocks db (H2 on cols):
                # wt[:, cb*1024 + c*256 + du] = xb0 +/- xb1, all flat
                wt = wt_pool.tile([128, 2048], BF16, name="wt")
                nc.vector.tensor_add(
                    wt[:, 0:1024], xb[:, 0:1024], xb[:, 1024:2048]
                )
                nc.vector.tensor_sub(
                    wt[:, 1024:2048], xb[:, 0:1024], xb[:, 1024:2048]
                )
                # MM1: z = (I2 (x) H256) w per row block mb (data stationary)
                # ps1[mb][p, db, dq, nu] = z[(mb, nu), (db, dq*128+p)]
                ps1 = [
                    psum_pool.tile([128, 1024], F32, name=f"ps1_{mb}", tag=f"ps1_{mb}")
                    for mb in range(2)
                ]
                for mb in range(2):
                    for cb in range(2):
                        for dq in range(2):
                            out = ps1[mb][
                                :, cb * 512 + dq * 256 : cb * 512 + (dq + 1) * 256
                            ]
                            for kq in range(2):
                                base = cb * 1024 + (2 * mb + kq) * 256 + dq * 128
                                nc.tensor.matmul(
                                    out,
                                    wt[:, base : base + 128],
                                    h_sb[:, kq * 256 : (kq + 1) * 256],
                                    start=(kq == 0),
                                    stop=(kq == 1),
                                )
                # C: psum -> sbuf bf16 relayout for MM2 lhsT (one op per mb)
                # zbig[p, db, dq, mb, nu] = z[(mb, nu), (db, dq*128+p)]
                zbig = zb_pool.tile([128, 2048], BF16, name="zbig")
                zv = zbig.rearrange("p (b q m u) -> p b q m u", b=2, q=2, m=2, u=256)
                p1v0 = ps1[0].rearrange("p (b q u) -> p b q u", b=2, q=2, u=256)
                p1v1 = ps1[1].rearrange("p (b q u) -> p b q u", b=2, q=2, u=256)
                nc.scalar.copy(zv[:, :, :, 0, :], p1v0[:, :, :, :])
                nc.scalar.copy(zv[:, :, :, 1, :], p1v1[:, :, :, :])
                return zbig

            def back_half(b, zbig):
                """MM2 + back butterfly + DMA out for item b."""
                # MM2: c = z (I2 (x) H256s) per col block db (data stationary)
                # ps2[mb][p, nq, db, eu] = c[(mb, nq*128+p), (db, eu)]
                ps2 = [
                    psum_pool.tile([128, 1024], F32, name=f"ps2_{mb}", tag=f"ps2_{mb}")
                    for mb in range(2)
                ]
                for db in range(2):
                    for mb in range(2):
                        for nq in range(2):
                            out = ps2[mb][
                                :, nq * 512 + db * 256 : nq * 512 + (db + 1) * 256
                            ]
                            for dq in range(2):
                                base = db * 1024 + dq * 512 + mb * 256 + nq * 128
                                nc.tensor.matmul(
                                    out,
                                    zbig[:, base : base + 128],
                                    hs_sb[:, dq * 256 : (dq + 1) * 256],
                                    start=(dq == 0),
                                    stop=(dq == 1),
                                )
                # E: back butterfly over row blocks (H2 on rows), bf16 out
                # yt[:, nb*1024 + (nq, db, eu)] = y[(nb, nq*128+p), (db,eu)]
                # only one PSUM operand allowed per instruction: stage ps2[1]
                # through SBUF, then butterfly PSUM(ps2[0]) +/- SBUF(c1)
                c1 = yt_pool.tile([128, 1024], BF16, name="c1", tag="c1")
                nc.scalar.copy(c1[:], ps2[1][:])
                yt = yt_pool.tile([128, 2048], BF16, name="yt")
                nc.vector.tensor_add(yt[:, 0:1024], ps2[0][:], c1[:])
                nc.vector.tensor_sub(yt[:, 1024:2048], ps2[0][:], c1[:])
                # DMA out in tile layout; host undoes the relayout.
                # Trigger from the (otherwise idle) gpsimd queue so input
                # DMA triggers on sync never queue behind output waits.
                nc.gpsimd.dma_start(y_d[b], yt[:])

            # software pipeline: back half runs one item behind the front
            # half so the PE never waits on the relayout copy C. Emit the
            # back half FIRST so early-ready ops (Ecopy/E of item i-1) are
            # not head-of-line blocked behind C(i) on the Act/DVE queues.
            prev = None
            for b in range(BPC):
                if prev is not None:
                    back_half(prev[0], prev[1])
                zbig = front_half(b)
                prev = (b, zbig)
            back_half(prev[0], prev[1])

    nc.compile()
    return nc


_NC = None


def _in_maps(x: np.ndarray) -> list[dict]:
    import ml_dtypes

    H = _hadamard(256)
    h_l = np.ascontiguousarray(
        H.reshape(2, 128, 256).transpose(1, 0, 2).reshape(128, 512)
    ).astype(ml_dtypes.bfloat16)
    hs_l = np.ascontiguousarray(
        (H / np.float32(512.0)).reshape(2, 128, 256).transpose(1, 0, 2).reshape(128, 512)
    ).astype(ml_dtypes.bfloat16)
    # relayout to tile form with db-halves contiguous:
    # x_t[b, p, db*1024 + c*256 + du] = x[b, c*128+p, db*256+du]
    x_t = np.ascontiguousarray(
        x.reshape(B, 4, 128, 2, 256)
        .transpose(0, 2, 3, 1, 4)
        .reshape(B, 128, 4 * D)
    ).astype(ml_dtypes.bfloat16)
    return [
        {"x": x_t[i * BPC : (i + 1) * BPC], "h": h_l, "hs": hs_l}
        for i in range(NCORES)
    ]


def _gather(results: list[dict]) -> np.ndarray:
    y_t = np.concatenate([r["y"] for r in results], axis=0)  # [B, 128, 2048]
    # y_t[b, p, (nb, nq, e)] = y[b, nb*256+nq*128+p, e]
    y = (
        y_t.reshape(B, 128, 4, D)
        .swapaxes(1, 2)
        .reshape(B, N, D)
        .astype(np.float32)
    )
    return np.ascontiguousarray(y)


def kernel(x: np.ndarray) -> np.ndarray:
    global _NC
    if _NC is None:
        _NC = _build()
    x = np.ascontiguousarray(np.asarray(x), dtype=np.float32)
    res = run_bass_kernel_spmd(_NC, _in_maps(x), list(range(NCORES))).results
    return _gather(res)


# revision 15
# speedup vs baseline: 5.4876x; 1.0206x over previous
"""Hadamard y = (1/512) H512 @ x @ H512 per batch item, 16 items/core.

Factorization: H512 = (H2 (x) I256) (I2 (x) H256)  [Sylvester, symmetric].
- column side: front butterfly over d-blocks (H2) + block-diag H256 matmul
- row side:    block-diag H256 matmul + back butterfly over row blocks (H2)
Tensor work/item: 32 MM of [128k,128m]x[128k,256n] (8192 rows) vs 16384
for full 512-matmuls. Butterflies ride DVE/Act. bf16 in/out (host casts).
"""
import sys

import numpy as np

for p in ("/opt/trn_rl_repo",):
    if p not in sys.path:
        sys.path.insert(0, p)

import concourse.bass as bass  # noqa: E402
import concourse.tile as tile  # noqa: E402
from concourse import bacc, mybir  # noqa: E402
from concourse.bass_utils import run_bass_kernel_spmd  # noqa: E402

B, N, D = 128, 512, 512
NCORES = 8
BPC = B // NCORES  # 16 batch items per core
F32 = mybir.dt.float32
BF16 = mybir.dt.bfloat16


def _hadamard(n: int) -> np.ndarray:
    H = np.array([[1.0]], dtype=np.float32)
    base = np.array([[1.0, 1.0], [1.0, -1.0]], dtype=np.float32)
    while H.shape[0] < n:
        H = np.kron(H, base)
    return H


def _build():
    nc = bacc.Bacc("TRN2", target_bir_lowering=False, debug=False)
    # x pre-relaid on host to the SBUF tile layout: x_t[b, p, c*512+d]
    # = x[b, c*128+p, d] -> one 4KB-per-partition DMA per item.
    x_d = nc.dram_tensor("x", [BPC, 128, 2048], BF16, kind="ExternalInput").ap()
    # H256 laid out [128, kq*256+nu] with row mu = kq*128+p
    h_d = nc.dram_tensor("h", [128, 512], BF16, kind="ExternalInput").ap()
    hs_d = nc.dram_tensor("hs", [128, 512], BF16, kind="ExternalInput").ap()
    # y left in tile layout: y_t[b, p, nb*1024+nq*512+e] = y[b, nb*256+nq*128+p, e]
    y_d = nc.dram_tensor("y", [BPC, 128, 2048], BF16, kind="ExternalOutput").ap()

    with tile.TileContext(nc) as tc:
        with (
            tc.tile_pool(name="const", bufs=1) as const_pool,
            tc.tile_pool(name="xb", bufs=3) as xb_pool,
            tc.tile_pool(name="wt", bufs=2) as wt_pool,
            tc.tile_pool(name="zb", bufs=2) as zb_pool,
            tc.tile_pool(name="yt", bufs=3) as yt_pool,
            tc.tile_pool(name="ps", bufs=1, space="PSUM") as psum_pool,
        ):
            h_sb = const_pool.tile([128, 512], BF16)
            nc.scalar.dma_start(h_sb[:], h_d[:])
            hs_sb = const_pool.tile([128, 512], BF16, tag="hs")
            nc.scalar.dma_start(hs_sb[:], hs_d[:])

            def front_half(b):
                """DMA in + front butterfly + MM1 + relayout copy -> zbig."""
                # xb[p, db*1024 + c*256 + du] = x[b, c*128+p, db*256+du];
                # row m=c*128+p (mb=c//2, mu-chunk kq=c%2); col d=(db, du).
                # db-halves contiguous so the butterfly uses flat APs (DVE
                # 16-bit fast mode needs contiguous patterns).
                xb = xb_pool.tile([128, 2048], BF16, name="xb")
                nc.sync.dma_start(xb[:], x_d[b])
                # A: front butterfly over column blocks db (H2 on cols):
                # wt[:, cb*1024 + c*256 + du] = xb0 +/- xb1, all flat
                wt = wt_pool.tile([128, 2048], BF16, name="wt")
                nc.vector.tensor_add(
                    wt[:, 0:1024], xb[:, 0:1024], xb[:, 1024:2048]
                )
                nc.vector.tensor_sub(
                    wt[:, 1024:2048], xb[:, 0:1024], xb[:, 1024:2048]
                )
                # MM1: z = (I2 (x) H256) w per row block mb (data stationary)
                # ps1[mb][p, db, dq, nu] = z[(mb, nu), (db, dq*128+p)]
                ps1 = [
                    psum_pool.tile([128, 1024], F32, name=f"ps1_{mb}", tag=f"ps1_{mb}")
                    for mb in range(2)
                ]
                for mb in range(2):
                    for cb in range(2):
                        for dq in range(2):
                            out = ps1[mb][
                                :, cb * 512 + dq * 256 : cb * 512 + (dq + 1) * 256
                            ]
                            for kq in range(2):
                                base = cb * 1024 + (2 * mb + kq) * 256 + dq * 128
                                nc.tensor.matmul(
                                    out,
                                    wt[:, base : base + 128],
                                    h_sb[:, kq * 256 : (kq + 1) * 256],
                                    start=(kq == 0),
                                    stop=(kq == 1),
                                )
                # C: psum -> sbuf bf16 relayout for MM2 lhsT (one op per mb)
                # zbig[p, db, dq, mb, nu] = z[(mb, nu), (db, dq*128+p)]
                zbig = zb_pool.tile([128, 2048], BF16, name="zbig")
                zv = zbig.rearrange("p (b q m u) -> p b q m u", b=2, q=2, m=2, u=256)
                p1v0 = ps1[0].rearrange("p (b q u) -> p b q u", b=2, q=2, u=256)
                p1v1 = ps1[1].rearrange("p (b q u) -> p b q u", b=2, q=2, u=256)
                nc.scalar.copy(zv[:, :, :, 0, :], p1v0[:, :, :, :])
                nc.scalar.copy(zv[:, :, :, 1, :], p1v1[:, :, :, :])
                return zbig

            def back_half(b, zbig):
                """MM2 + back butterfly + DMA out for item b."""
                # MM2: c = z (I2 (x) H256s) per col block db (data stationary)
                # ps2[mb][p, nq, db, eu] = c[(mb, nq*128+p), (db, eu)]
                ps2 = [
                    psum_pool.tile([128, 1024], F32, name=f"ps2_{mb}", tag=f"ps2_{mb}")
                    for mb in range(2)
                ]
                for mb in (1, 0):
                    for db in range(2):
                        for nq in range(2):
                            out = ps2[mb][
                                :, nq * 512 + db * 256 : nq * 512 + (db + 1) * 256
                            ]
                            for dq in range(2):
                                base = db * 1024 + dq * 512 + mb * 256 + nq * 128
                                nc.tensor.matmul(
                                    out,
                                    zbig[:, base : base + 128],
                                    hs_sb[:, dq * 256 : (dq + 1) * 256],
                                    start=(dq == 0),
                                    stop=(dq == 1),
                                )
                # E: back butterfly over row blocks (H2 on rows), bf16 out
                # yt[:, nb*1024 + (nq, db, eu)] = y[(nb, nq*128+p), (db,eu)]
                # only one PSUM operand allowed per instruction: stage ps2[1]
                # through SBUF, then butterfly PSUM(ps2[0]) +/- SBUF(c1)
                c1 = yt_pool.tile([128, 1024], BF16, name="c1", tag="c1")
                nc.scalar.copy(c1[:], ps2[1][:])
                yt = yt_pool.tile([128, 2048], BF16, name="yt")
                nc.vector.tensor_add(yt[:, 0:1024], ps2[0][:], c1[:])
                if b == BPC - 1:
                    # drain tail: ship the first half while the subtract
                    # of the second half still runs
                    nc.gpsimd.dma_start(y_d[b][:, 0:1024], yt[:, 0:1024])
                    nc.vector.tensor_sub(yt[:, 1024:2048], ps2[0][:], c1[:])
                    nc.gpsimd.dma_start(y_d[b][:, 1024:2048], yt[:, 1024:2048])
                else:
                    nc.vector.tensor_sub(yt[:, 1024:2048], ps2[0][:], c1[:])
                    # DMA out in tile layout; host undoes the relayout.
                    # Trigger from the (otherwise idle) gpsimd queue so input
                    # DMA triggers on sync never queue behind output waits.
                    nc.gpsimd.dma_start(y_d[b], yt[:])

            # software pipeline: back half runs one item behind the front
            # half so the PE never waits on the relayout copy C. Emit the
            # back half FIRST so early-ready ops (Ecopy/E of item i-1) are
            # not head-of-line blocked behind C(i) on the Act/DVE queues.
            prev = None
            for b in range(BPC):
                if prev is not None:
                    back_half(prev[0], prev[1])
                zbig = front_half(b)
                prev = (b, zbig)
            back_half(prev[0], prev[1])

    nc.compile()
    return nc


_NC = None


def _in_maps(x: np.ndarray) -> list[dict]:
    import ml_dtypes

    H = _hadamard(256)
    h_l = np.ascontiguousarray(
        H.reshape(2, 128, 256).transpose(1, 0, 2).reshape(128, 512)
    ).astype(ml_dtypes.bfloat16)
    hs_l = np.ascontiguousarray(
        (H / np.float32(512.0)).reshape(2, 128, 256).transpose(1, 0, 2).reshape(128, 512)
    ).astype(ml_dtypes.bfloat16)
    # relayout to tile form with db-halves contiguous:
    # x_t[b, p, db*1024 + c*256 + du] = x[b, c*128+p, db*256+du]
    x_t = np.ascontiguousarray(
        x.reshape(B, 4, 128, 2, 256)
        .transpose(0, 2, 3, 1, 4)
        .reshape(B, 128, 4 * D)
    ).astype(ml_dtypes.bfloat16)
    return [
        {"x": x_t[i * BPC : (i + 1) * BPC], "h": h_l, "hs": hs_l}
        for i in range(NCORES)
    ]


def _gather(results: list[dict]) -> np.ndarray:
    y_t = np.concatenate([r["y"] for r in results], axis=0)  # [B, 128, 2048]
    # y_t[b, p, (nb, nq, e)] = y[b, nb*256+nq*128+p, e]
    y = (
        y_t.reshape(B, 128, 4, D)
        .swapaxes(1, 2)
        .reshape(B, N, D)
        .astype(np.float32)
    )
    return np.ascontiguousarray(y)


def kernel(x: np.ndarray) -> np.ndarray:
    global _NC
    if _NC is None:
        _NC = _build()
    x = np.ascontiguousarray(np.asarray(x), dtype=np.float32)
    res = run_bass_kernel_spmd(_NC, _in_maps(x), list(range(NCORES))).results
    return _gather(res)
